# revision 24
# baseline (speedup 1.0000x reference)
"""Trainium2 Bass kernel for nn_MultiHeadAttention_56375740727430.

Causal multi-head attention, B=2 S=2048 D=1024 H=16 KS=64, followed by an
output projection `heads @ kernel`.

Sharding: pure data/head parallel over 8 cores — core c handles batch c//4
and 4 heads (c%4)*4 ... +4.  Each core computes Q^T/K^T (head-pair-stacked,
transposed layout), V (natural layout, with an appended ones-column so the
softmax denominator Z falls out of the attention matmul), causal scores ->
exp -> (P@V | Z) -> per-head output projection, all unnormalized.  The host
divides by Z, sums head contributions and batch-partials, and transposes.

Performance structure:
- fp8e4 DoubleRow (K=256/instruction) for the QKV projections and for
  off-diagonal P@V pairs; exp writes e4m3 directly with a ln(8) bias that
  cancels in the host-side Z normalization.
- Hybrid precision: softmax averaging shrinks fp8 noise by ~1/sqrt(row
  support), so fp8 error only survives in early query rows.  Everything
  feeding rows i<512 (projection i-block 0, V tiles 0-3, attention block
  ib=0) therefore runs in bf16; W is host-prescaled by 64 in both dtypes
  (folded into the exp scale / V copy) to keep fp8 W out of e4m3's
  subnormal range.
- Scores stay bf16 (fp8 gains nothing at K=64; keeps exp input accurate);
  the two heads of a pair run concurrently via PE row tiling.
- x is DMA'd in (t, i-block) chunks and emission interleaves projections
  with attention blocks so exp (the scalar-engine bottleneck) starts early
  and runs back-to-back.
"""

import sys

sys.path.insert(0, "/opt/trn_rl_repo")

import math
import os
from contextlib import ExitStack

import ml_dtypes
import numpy as np

import concourse.bass as bass
import concourse.bacc as bacc
import concourse.mybir as mybir
import concourse.tile as tile

B, S, D = 2, 2048, 1024
H, KS = 16, 64

P = 128            # partitions
NCORES = 8
CORES_PER_B = NCORES // B          # 4
NH = H // CORES_PER_B              # heads per core = 4
NW = NH * KS                       # per-core projection width = 256
DT = D // P                        # d-tiles = 8
ST = S // P                        # s/l-tiles = 16
IB = 512                           # query block
NIB = S // IB                      # 4
LPB = IB // P                      # l-tiles per query block = 4

PROJ_FP8 = os.environ.get("K_PROJ_FP8", "1") == "1"  # QKV proj fp8 (ic>0)
PV_FP8 = os.environ.get("K_PV_FP8", "1") == "1"      # P@V fp8 (ib>0)
MASK_GP = os.environ.get("K_MASK_GP", "1") == "1"    # masks on GpSimd
W_SCALE = 64.0                     # host prescale on Wq/Wk/Wv
PE_BIAS = math.log(8.0)            # exp bias on fp8 blocks (cancels via Z)
KSP = 68                           # padded V row count in fp8 mode (64+z+pad)

F32 = mybir.dt.float32
BF16 = mybir.dt.bfloat16
FP8 = mybir.dt.float8e4
NP_BF16 = ml_dtypes.bfloat16
NP_FP8 = ml_dtypes.float8_e4m3
EXP = mybir.ActivationFunctionType.Exp
DR = mybir.MatmulPerfMode.DoubleRow

EXP_SCALE = 0.125 / (W_SCALE * W_SCALE)


def build_nc():
    nc = bacc.Bacc()

    # bf16 x/weights feed the i<512 (bf16) blocks; fp8 copies feed the rest
    xT0 = nc.declare_dram_parameter("xT0", [D, S], BF16, isOutput=False)
    xT = nc.declare_dram_parameter("xT", [D, S], FP8, isOutput=False)
    # weights host-packed to the SBUF layout so their DMA is contiguous
    wq = nc.declare_dram_parameter("wq", [P, DT * NW], BF16, isOutput=False)
    wk = nc.declare_dram_parameter("wk", [P, DT * NW], BF16, isOutput=False)
    wv = nc.declare_dram_parameter("wv", [P, DT * NW], BF16, isOutput=False)
    wkern = nc.declare_dram_parameter("wkern", [KS, NH * KS], BF16, isOutput=False)
    masks = nc.declare_dram_parameter("masks", [P, P], BF16, isOutput=False)
    outT = nc.declare_dram_parameter("outT", [NH, KS, S], F32, isOutput=True)
    z = nc.declare_dram_parameter("z", [NH, S], F32, isOutput=True)

    with tile.TileContext(nc) as tc, ExitStack() as ctx:
        const_pool = ctx.enter_context(tc.tile_pool(name="const", bufs=1))
        qkv_pool = ctx.enter_context(tc.tile_pool(name="qkv", bufs=1))
        out_pool = ctx.enter_context(tc.tile_pool(name="outp", bufs=1))
        xw_pool = ctx.enter_context(tc.tile_pool(name="xw", bufs=1))
        pexp_pool = ctx.enter_context(tc.tile_pool(name="pexp", bufs=6))
        osb_pool = ctx.enter_context(tc.tile_pool(name="osb", bufs=4))
        pp = ctx.enter_context(
            tc.tile_pool(name="pp", bufs=1, space=bass.MemorySpace.PSUM)
        )
        pst = ctx.enter_context(
            tc.tile_pool(name="pst", bufs=2, space=bass.MemorySpace.PSUM)
        )
        po = ctx.enter_context(
            tc.tile_pool(name="po", bufs=1, space=bass.MemorySpace.PSUM)
        )
        pwarm = ctx.enter_context(
            tc.tile_pool(name="pwarm", bufs=1, space=bass.MemorySpace.PSUM)
        )

        # PE warmup: dependency-free matmuls on zeroed scratch so the HAM
        # clock gate reaches 8/8 during the input-DMA lead-in
        # one dedicated PSUM bank, written by filler matmuls and never read:
        # fillers absorb PE slack during exp-bound stretches so the HAM clock
        # gate never re-throttles the PE to 1.2 GHz
        warm_in = const_pool.tile([P, IB], BF16)
        nc.vector.memset(warm_in[:], 0.0)
        warm_ps = pwarm.tile([P, IB], F32, tag="warm")

        def filler(n=IB):
            nc.tensor.matmul(
                warm_ps[:, 0:n], warm_in[:, 0:P], warm_in[:, 0:n],
                start=True, stop=True, skip_group_check=True,
            )

        for wi in range(36):
            filler(2 * P)

        wkern_sb = const_pool.tile([KS, NH, KS], BF16)
        nc.sync.dma_start(
            wkern_sb[:], wkern[:].rearrange("k (h j) -> k h j", j=KS)
        )
        mask_sb = const_pool.tile([P, P], BF16)
        nc.sync.dma_start(mask_sb[:], masks[:])
        bias_sb = const_pool.tile([P, 1], F32)
        nc.vector.memset(bias_sb[:], PE_BIAS)

        w_sb, w8_sb = {}, {}

        def load_w(name, wh):
            w_sb[name] = xw_pool.tile(
                [P, DT, NW], BF16, tag=f"w{name}", name=f"w{name}"
            )
            nc.sync.dma_start(
                w_sb[name][:], wh[:].rearrange("p (t n) -> p t n", n=NW)
            )
            if PROJ_FP8:
                w8_sb[name] = xw_pool.tile(
                    [P, DT, NW], FP8, tag=f"w8{name}", name=f"w8{name}"
                )
                nc.vector.tensor_copy(w8_sb[name][:], w_sb[name][:])

        # bf16 x covers only the i<512 block in fp8 mode, all of x otherwise
        XB = IB if PROJ_FP8 else S
        load_w("q", wq)
        xb_sb = xw_pool.tile([P, DT, XB], BF16, tag="xb")
        for ic in range(XB // IB):
            for t in range(DT):
                nc.sync.dma_start(
                    xb_sb[:, t, ic * IB : (ic + 1) * IB],
                    xT0[t * P : (t + 1) * P, ic * IB : (ic + 1) * IB],
                )
        load_w("k", wk)
        load_w("v", wv)
        # fp8 x in (t, i-block) chunks, i-block major; the i<512 chunk is
        # only needed in bf16 (nothing reads fp8 x there)
        if PROJ_FP8:
            xT_sb = xw_pool.tile([P, DT, S], FP8, tag="xT")
            for ic in range(1, NIB):
                for t in range(DT):
                    nc.sync.dma_start(
                        xT_sb[:, t, ic * IB : (ic + 1) * IB],
                        xT[t * P : (t + 1) * P, ic * IB : (ic + 1) * IB],
                    )

        qt_sb = [
            qkv_pool.tile([P, S], BF16, tag=f"qt{i}", name=f"qt{i}") for i in range(2)
        ]
        kt_sb = [
            qkv_pool.tile([P, S], BF16, tag=f"kt{i}", name=f"kt{i}") for i in range(2)
        ]
        # bf16 V (ones col at 64) for the ib=0 attention block
        v0_sb = qkv_pool.tile([P, LPB, NH, KS + 1], BF16, tag="v0")
        nc.vector.memset(v0_sb[:, :, :, KS], 1.0)
        if PV_FP8:
            # fp8 V in DoubleRow-friendly (l-tile-pair, parity) layout
            v_sb = qkv_pool.tile([P, ST // 2, 2, NH, KSP], FP8, tag="v")
            nc.vector.memset(v_sb[:, :, :, :, KS], 1.0)
            nc.vector.memset(v_sb[:, :, :, :, KS + 1 : KSP], 0.0)
        else:
            v_sb = qkv_pool.tile([P, ST, NH, KS + 1], BF16, tag="v")
            nc.vector.memset(v_sb[:, :, :, KS], 1.0)
        outT_sb = out_pool.tile([KS, NH, S], F32)

        def mm_acc(ps, fp8, lhsT_of, rhs_of):
            """Full-D contraction: fp8 DoubleRow (4x K=256) or bf16 (8x)."""
            if fp8:
                for u in range(DT // 2):
                    nc.tensor.matmul(
                        ps, lhsT_of(2 * u, 2), rhs_of(2 * u, 2),
                        start=(u == 0), stop=(u == DT // 2 - 1),
                        perf_mode=DR,
                    )
            else:
                for t in range(DT):
                    nc.tensor.matmul(
                        ps, lhsT_of(t, 1), rhs_of(t, 1),
                        start=(t == 0), stop=(t == DT - 1),
                    )

        def proj_qk(pt, ic):
            # Q^T / K^T for head-pair pt, i-block ic: [n, s] layout
            fp8 = PROJ_FP8 and ic > 0
            wt, xt = (w8_sb, xT_sb) if fp8 else (w_sb, xb_sb)
            x0 = ic * IB if (fp8 or not PROJ_FP8) else 0  # xb holds ic=0 only
            for wname, dst in (("q", qt_sb), ("k", kt_sb)):
                ps = pp.tile([P, IB], F32, tag="of", name="ps")
                mm_acc(
                    ps[:], fp8,
                    lambda t, m, w=wname: wt[w][:, t : t + m, pt * P : (pt + 1) * P],
                    lambda t, m: xt[:, t : t + m, x0 : x0 + IB],
                )
                nc.vector.tensor_copy(dst[pt][:, ic * IB : (ic + 1) * IB], ps[:])

        def proj_v(st):
            # V: natural [s, n] layout, all heads, with ones column
            fp8 = PROJ_FP8 and st >= LPB
            wt, xt = (w8_sb, xT_sb) if fp8 else (w_sb, xb_sb)
            x0 = st * P  # st < 4 lies inside xb's i<512 window in fp8 mode
            ps = pp.tile([P, NW], F32, tag="of", name="ps")
            mm_acc(
                ps[:], fp8,
                lambda t, m: xt[:, t : t + m, x0 : x0 + P],
                lambda t, m: wt["v"][:, t : t + m, :],
            )
            src = ps[:].rearrange("p (h k) -> p h k", k=KS)
            if st < LPB:
                nc.vector.tensor_scalar_mul(
                    v0_sb[:, st, :, 0:KS], src, 1.0 / W_SCALE
                )
            if PV_FP8:
                dst = v_sb[:, st // 2, st % 2, :, 0:KS]
            else:
                dst = v_sb[:, st, :, 0:KS]
            nc.vector.tensor_scalar_mul(dst, src, 1.0 / W_SCALE)

        def attention_ib(pr, ib):
            # causal attention + output projection for head pair pr, i-block
            # ib; scores row-packed via tile_position so both heads' K=64
            # matmuls run concurrently on the PE array
            fp8 = PV_FP8 and ib > 0
            nl = (ib + 1) * LPB
            ndiag = ib * LPB  # l-tiles before the diagonal block (off == 0)
            o_ps = [
                po.tile([KSP, IB], F32, tag=f"o{hh}", name=f"o{pr}_{ib}_{hh}")
                for hh in range(2)
            ]
            pe_t = None
            for lt in range(nl):
                off = max(0, (lt - ndiag)) * P
                st_ps = pst.tile([P, 2, IB], F32, tag="st", name="st")
                for hh in range(2):
                    nc.tensor.matmul(
                        st_ps[:, hh, off:IB],
                        kt_sb[pr][hh * KS : (hh + 1) * KS, lt * P : (lt + 1) * P],
                        qt_sb[pr][
                            hh * KS : (hh + 1) * KS,
                            ib * IB + off : (ib + 1) * IB,
                        ],
                        start=True,
                        stop=True,
                        tile_position=(hh * KS, 0),
                    )
                if fp8:
                    if lt % 2 == 0:
                        pe_t = pexp_pool.tile(
                            [P, 2, 2, IB], FP8, tag="pe", name="pe"
                        )
                    pe_sl = pe_t[:, :, lt % 2, :]
                    nc.scalar.activation(
                        pe_sl[:, :, off:IB], st_ps[:, :, off:IB], EXP,
                        scale=EXP_SCALE, bias=bias_sb[:],
                    )
                else:
                    pe_t = pexp_pool.tile([P, 2, IB], BF16, tag="pe0", name="pe")
                    pe_sl = pe_t[:, :, :]
                    nc.scalar.activation(
                        pe_sl[:, :, off:IB], st_ps[:, :, off:IB], EXP,
                        scale=EXP_SCALE,
                    )
                if lt >= ndiag:  # diagonal 128-block -> triangular mask
                    # on GpSimd (otherwise idle) to keep DVE off the floor
                    for hh in range(2):
                        MASK_ENG.tensor_mul(
                            pe_sl[:, hh, off : off + P],
                            pe_sl[:, hh, off : off + P],
                            mask_sb[:],
                        )
                # P@V accumulation.  Diagonal tiles are split into the
                # masked 128-col strip and the mask-free tail so the bulk of
                # P@V doesn't wait on the mask multiply.
                if fp8:
                    if lt < ndiag and lt % 2 == 1:
                        # completed off-diagonal pair: DoubleRow, K=256
                        for hh in range(2):
                            nc.tensor.matmul(
                                o_ps[hh][:],
                                v_sb[:, lt // 2, :, 2 * pr + hh, :],
                                pe_t[:, hh, :, :],
                                start=(lt == 1), stop=False,
                                perf_mode=DR,
                            )
                    elif lt >= ndiag:
                        for hh in range(2):
                            vsl = v_sb[:, lt // 2, lt % 2, 2 * pr + hh, :]
                            if off + P < IB:
                                nc.tensor.matmul(
                                    o_ps[hh][:, off + P : IB],
                                    vsl,
                                    pe_t[:, hh, lt % 2, off + P : IB],
                                    start=(lt == 0), stop=False,
                                )
                            nc.tensor.matmul(
                                o_ps[hh][:, off : off + P],
                                vsl,
                                pe_t[:, hh, lt % 2, off : off + P],
                                start=(lt == 0 and off + P >= IB),
                                stop=(lt == nl - 1),
                            )
                else:
                    for hh in range(2):
                        vsl = (
                            v0_sb[:, lt, 2 * pr + hh, :]
                            if ib == 0
                            else v_sb[:, lt, 2 * pr + hh, :]
                        )
                        if lt >= ndiag:
                            if off + P < IB:
                                nc.tensor.matmul(
                                    o_ps[hh][0 : KS + 1, off + P : IB],
                                    vsl,
                                    pe_t[:, hh, off + P : IB],
                                    start=(lt == 0), stop=False,
                                )
                            nc.tensor.matmul(
                                o_ps[hh][0 : KS + 1, off : off + P],
                                vsl,
                                pe_t[:, hh, off : off + P],
                                start=(lt == 0 and off + P >= IB),
                                stop=(lt == nl - 1),
                            )
                        else:
                            nc.tensor.matmul(
                                o_ps[hh][0 : KS + 1, off:IB],
                                vsl,
                                pe_t[:, hh, off:IB],
                                start=(lt == 0), stop=(lt == nl - 1),
                            )
                filler()  # keep the PE dense through the exp-bound stretch
                if lt % 2 == 0:
                    filler()
            for hh in range(2):
                h = 2 * pr + hh
                # bf16 rows for the projection matmul, f32 Z row for
                # exact normalization on the host
                o_bf = osb_pool.tile([KS, IB], BF16, tag="o_bf", name="o_bf")
                nc.vector.tensor_copy(o_bf[:], o_ps[hh][0:KS, :])
                z_sb = osb_pool.tile([1, IB], F32, tag="z_sb", name="z_sb")
                nc.vector.tensor_copy(z_sb[:], o_ps[hh][KS : KS + 1, :])
                nc.sync.dma_start(z[h, ib * IB : (ib + 1) * IB], z_sb[:])
                f_ps = pp.tile([KS, IB], F32, tag="of", name="f_ps")
                nc.tensor.matmul(
                    f_ps[:], wkern_sb[:, h, :], o_bf[:], start=True, stop=True
                )
                nc.vector.tensor_copy(
                    outT_sb[:, h, ib * IB : (ib + 1) * IB], f_ps[:]
                )
            nc.sync.dma_start(
                outT[:].rearrange("h k s -> k h s")[
                    :, 2 * pr : 2 * pr + 2, ib * IB : (ib + 1) * IB
                ],
                outT_sb[:, 2 * pr : 2 * pr + 2, ib * IB : (ib + 1) * IB],
            )

        MASK_ENG = nc.gpsimd if MASK_GP else nc.vector

        # global interleave: attention blocks (the scalar-engine exp stream)
        # run back-to-back across BOTH head pairs while projection matmuls
        # fill the PE between them — exp never starves, PE never idles long
        # enough to re-throttle the HAM clock gate
        proj_qk(0, 0)
        for st in range(0, LPB):
            proj_v(st)
        attention_ib(0, 0)
        proj_qk(1, 0)
        for ib in range(1, NIB):
            proj_qk(0, ib)
            for st in range(ib * LPB, (ib + 1) * LPB):
                proj_v(st)
            attention_ib(1, ib - 1)
            attention_ib(0, ib)
            proj_qk(1, ib)
        attention_ib(1, NIB - 1)

    nc.compile()
    return nc


def make_masks():
    # triangular [P, P]: within a diagonal 128-block keep j >= p
    j = np.arange(P)[None, :]
    p = np.arange(P)[:, None]
    return (j >= p).astype(NP_BF16)


def make_in_maps(inputs):
    x = np.asarray(inputs["x"], np.float32)
    Wq = np.asarray(inputs["Wq"], np.float32)
    Wk = np.asarray(inputs["Wk"], np.float32)
    Wv = np.asarray(inputs["Wv"], np.float32)
    kern = np.asarray(inputs["kernel"], np.float32)

    masks = make_masks()
    kern3 = kern.reshape(KS, H, KS)  # [k, h, j]

    def packw(W, hs):
        Wp = W[:, :, hs : hs + NH].transpose(0, 2, 1).reshape(D, NW) * W_SCALE
        # pre-pack to the SBUF [P, DT, NW] layout (row-major flattened)
        return (
            Wp.reshape(DT, P, NW).transpose(1, 0, 2).reshape(P, DT * NW)
            .astype(NP_BF16)
        )

    in_maps = []
    for c in range(NCORES):
        b, hs = c // CORES_PER_B, (c % CORES_PER_B) * NH
        xb = x[b].T  # [D, S]
        in_maps.append(
            {
                "xT0": xb.astype(NP_BF16),
                "xT": np.clip(xb, -240, 240).astype(NP_FP8),
                "wq": packw(Wq, hs),
                "wk": packw(Wk, hs),
                "wv": packw(Wv, hs),
                "wkern": kern3[:, hs : hs + NH, :].reshape(KS, NH * KS)
                .astype(NP_BF16),
                "masks": masks,
            }
        )
    return in_maps


def gather_output(results):
    out = np.zeros((B, S, KS), np.float32)
    for c in range(NCORES):
        b = c // CORES_PER_B
        oT = np.asarray(results[c]["outT"], np.float32)  # [NH, KS, S]
        zz = np.asarray(results[c]["z"], np.float32)     # [NH, S]
        out[b] += (oT / zz[:, None, :]).sum(axis=0).T
    return out


_NC_CACHE = {}


def get_nc():
    if "nc" not in _NC_CACHE:
        _NC_CACHE["nc"] = build_nc()
    return _NC_CACHE["nc"]


def run_hw(inputs, trace=False, **kw):
    from concourse.bass_utils import run_bass_kernel_spmd

    nc = get_nc()
    in_maps = make_in_maps(inputs)
    res = run_bass_kernel_spmd(
        nc, in_maps, list(range(NCORES)), trace=trace, **kw
    )
    return gather_output(res.results), res


def kernel(**inputs) -> np.ndarray:
    out, _ = run_hw(inputs, trace=False)
    return out


# revision 46
# speedup vs baseline: 1.2591x; 1.2591x over previous
"""Trainium2 Bass kernel for nn_MultiHeadAttention_56375740727430.

Causal multi-head attention, B=2 S=2048 D=1024 H=16 KS=64, followed by an
output projection `heads @ kernel`.

Sharding: pure data/head parallel over 8 cores — core c handles batch c//4
and 4 heads (c%4)*4 ... +4.  Each core computes Q^T/K^T (head-pair-stacked,
transposed layout), V (natural layout, with an appended ones-column so the
softmax denominator Z falls out of the attention matmul), causal scores ->
exp -> (P@V | Z) -> per-head output projection, all unnormalized.  The host
divides by Z, sums head contributions and batch-partials, and transposes.

Performance structure:
- fp8e4 DoubleRow (K=256/instruction) for the QKV projections and for
  off-diagonal P@V pairs; exp writes e4m3 directly with a ln(8) bias that
  cancels in the host-side Z normalization.
- Hybrid precision: softmax averaging shrinks fp8 noise by ~1/sqrt(row
  support), so fp8 error only survives in early query rows.  Everything
  feeding rows i<512 (projection i-block 0, V tiles 0-3, attention block
  ib=0) therefore runs in bf16; W is host-prescaled by 64 in both dtypes
  (folded into the exp scale / V copy) to keep fp8 W out of e4m3's
  subnormal range.
- Scores stay bf16 (fp8 gains nothing at K=64; keeps exp input accurate);
  the two heads of a pair run concurrently via PE row tiling.
- x is DMA'd in (t, i-block) chunks and emission interleaves projections
  with attention blocks so exp (the scalar-engine bottleneck) starts early
  and runs back-to-back.
"""

import sys

sys.path.insert(0, "/opt/trn_rl_repo")

import math
import os
from contextlib import ExitStack

import ml_dtypes
import numpy as np

import concourse.bass as bass
import concourse.bacc as bacc
import concourse.mybir as mybir
import concourse.tile as tile

B, S, D = 2, 2048, 1024
H, KS = 16, 64

P = 128            # partitions
NCORES = 8
CORES_PER_B = NCORES // B          # 4
NH = H // CORES_PER_B              # heads per core = 4
NW = NH * KS                       # per-core projection width = 256
DT = D // P                        # d-tiles = 8
ST = S // P                        # s/l-tiles = 16
IB = 512                           # query block
NIB = S // IB                      # 4
LPB = IB // P                      # l-tiles per query block = 4

PROJ_FP8 = os.environ.get("K_PROJ_FP8", "1") == "1"  # QKV proj fp8 (ic>0)
PV_FP8 = os.environ.get("K_PV_FP8", "1") == "1"      # P@V fp8 (ib>0)
MASK_GP = os.environ.get("K_MASK_GP", "1") == "1"    # masks on GpSimd
W_SCALE = 64.0                     # host prescale on Wq/Wk/Wv
PE_BIAS = math.log(8.0)            # exp bias on fp8 blocks (cancels via Z)
KSP = 68                           # padded V row count in fp8 mode (64+z+pad)

F32 = mybir.dt.float32
BF16 = mybir.dt.bfloat16
FP8 = mybir.dt.float8e4
NP_BF16 = ml_dtypes.bfloat16
NP_FP8 = ml_dtypes.float8_e4m3
EXP = mybir.ActivationFunctionType.Exp
DR = mybir.MatmulPerfMode.DoubleRow

EXP_SCALE = 0.125 / (W_SCALE * W_SCALE)


def build_nc():
    nc = bacc.Bacc()

    # bf16 x/weights feed the i<512 (bf16) blocks; fp8 copies feed the rest
    xT0 = nc.declare_dram_parameter("xT0", [D, S], BF16, isOutput=False)
    xT = nc.declare_dram_parameter("xT", [D, S], FP8, isOutput=False)
    # weights host-packed to the SBUF layout so their DMA is contiguous
    wq = nc.declare_dram_parameter("wq", [P, DT * NW], BF16, isOutput=False)
    wk = nc.declare_dram_parameter("wk", [P, DT * NW], BF16, isOutput=False)
    wv = nc.declare_dram_parameter("wv", [P, DT * NW], BF16, isOutput=False)
    wkern = nc.declare_dram_parameter("wkern", [KS, NH * KS], BF16, isOutput=False)
    masks = nc.declare_dram_parameter("masks", [P, P], BF16, isOutput=False)
    outT = nc.declare_dram_parameter("outT", [NH, KS, S], BF16, isOutput=True)
    z = nc.declare_dram_parameter("z", [NH, S], F32, isOutput=True)

    with tile.TileContext(nc) as tc, ExitStack() as ctx:
        const_pool = ctx.enter_context(tc.tile_pool(name="const", bufs=1))
        qkv_pool = ctx.enter_context(tc.tile_pool(name="qkv", bufs=1))
        out_pool = ctx.enter_context(tc.tile_pool(name="outp", bufs=1))
        xw_pool = ctx.enter_context(tc.tile_pool(name="xw", bufs=1))
        pexp_pool = ctx.enter_context(tc.tile_pool(name="pexp", bufs=6))
        osb_pool = ctx.enter_context(tc.tile_pool(name="osb", bufs=4))
        pp = ctx.enter_context(
            tc.tile_pool(name="pp", bufs=1, space=bass.MemorySpace.PSUM)
        )
        pst = ctx.enter_context(
            tc.tile_pool(name="pst", bufs=2, space=bass.MemorySpace.PSUM)
        )
        po = ctx.enter_context(
            tc.tile_pool(name="po", bufs=1, space=bass.MemorySpace.PSUM)
        )
        pwarm = ctx.enter_context(
            tc.tile_pool(name="pwarm", bufs=1, space=bass.MemorySpace.PSUM)
        )

        # PE warmup: dependency-free matmuls on zeroed scratch so the HAM
        # clock gate reaches 8/8 during the input-DMA lead-in
        # one dedicated PSUM bank, written by filler matmuls and never read:
        # fillers absorb PE slack during exp-bound stretches so the HAM clock
        # gate never re-throttles the PE to 1.2 GHz
        warm_in = const_pool.tile([P, IB], BF16)
        nc.vector.memset(warm_in[:], 0.0)
        warm_ps = pwarm.tile([P, IB], F32, tag="warm")

        def filler(n=IB):
            nc.tensor.matmul(
                warm_ps[:, 0:n], warm_in[:, 0:P], warm_in[:, 0:n],
                start=True, stop=True, skip_group_check=True,
            )

        for wi in range(20):
            filler(2 * P)

        wkern_sb = const_pool.tile([KS, NH, KS], BF16)
        nc.sync.dma_start(
            wkern_sb[:], wkern[:].rearrange("k (h j) -> k h j", j=KS)
        )
        mask_sb = const_pool.tile([P, P], BF16)
        nc.sync.dma_start(mask_sb[:], masks[:])
        bias_sb = const_pool.tile([P, 1], F32)
        nc.vector.memset(bias_sb[:], PE_BIAS)

        w_sb, w8_sb = {}, {}

        def load_w(name, wh):
            w_sb[name] = xw_pool.tile(
                [P, DT, NW], BF16, tag=f"w{name}", name=f"w{name}"
            )
            nc.sync.dma_start(
                w_sb[name][:], wh[:].rearrange("p (t n) -> p t n", n=NW)
            )
            if PROJ_FP8:
                w8_sb[name] = xw_pool.tile(
                    [P, DT, NW], FP8, tag=f"w8{name}", name=f"w8{name}"
                )
                nc.vector.tensor_copy(w8_sb[name][:], w_sb[name][:])

        # bf16 x covers only the i<512 block in fp8 mode, all of x otherwise
        XB = IB if PROJ_FP8 else S
        load_w("q", wq)
        xb_sb = xw_pool.tile([P, DT, XB], BF16, tag="xb")
        for ic in range(XB // IB):
            for t in range(DT):
                nc.sync.dma_start(
                    xb_sb[:, t, ic * IB : (ic + 1) * IB],
                    xT0[t * P : (t + 1) * P, ic * IB : (ic + 1) * IB],
                )
        load_w("k", wk)
        load_w("v", wv)
        # fp8 x in (t, i-block) chunks, i-block major; the i<512 chunk is
        # only needed in bf16 (nothing reads fp8 x there)
        if PROJ_FP8:
            xT_sb = xw_pool.tile([P, DT, S], FP8, tag="xT")
            for ic in range(1, NIB):
                for t in range(DT):
                    nc.sync.dma_start(
                        xT_sb[:, t, ic * IB : (ic + 1) * IB],
                        xT[t * P : (t + 1) * P, ic * IB : (ic + 1) * IB],
                    )

        qt_sb = [
            qkv_pool.tile([P, S], BF16, tag=f"qt{i}", name=f"qt{i}") for i in range(2)
        ]
        kt_sb = [
            qkv_pool.tile([P, S], BF16, tag=f"kt{i}", name=f"kt{i}") for i in range(2)
        ]
        # bf16 V (ones col at 64) for the ib=0 attention block
        v0_sb = qkv_pool.tile([P, LPB, NH, KS + 1], BF16, tag="v0")
        nc.vector.memset(v0_sb[:, :, :, KS], 1.0)
        if PV_FP8:
            # fp8 V in DoubleRow-friendly (l-tile-pair, parity) layout
            v_sb = qkv_pool.tile([P, ST // 2, 2, NH, KSP], FP8, tag="v")
            nc.vector.memset(v_sb[:, :, :, :, KS], 1.0)
            nc.vector.memset(v_sb[:, :, :, :, KS + 1 : KSP], 0.0)
        else:
            v_sb = qkv_pool.tile([P, ST, NH, KS + 1], BF16, tag="v")
            nc.vector.memset(v_sb[:, :, :, KS], 1.0)
        outT_sb = out_pool.tile([KS, NH, S], BF16)

        # Projections are emitted as single-matmul "quanta", pumped a few at
        # a time into the attention pipeline's slack so the exp stream (the
        # scalar-engine bottleneck) never breaks and the PE never idles.
        def gen_proj_qk(pt, ic):
            fp8 = PROJ_FP8 and ic > 0
            wt, xt = (w8_sb, xT_sb) if fp8 else (w_sb, xb_sb)
            x0 = ic * IB if (fp8 or not PROJ_FP8) else 0  # xb holds ic=0 only
            nu = DT // 2 if fp8 else DT
            for wname, dst in (("q", qt_sb), ("k", kt_sb)):
                st8 = {}
                for u in range(nu):
                    def q(u=u, wname=wname, dst=dst, st8=st8):
                        if u == 0:
                            st8["ps"] = pp.tile([P, IB], F32, tag="of", name="ps")
                        ps = st8["ps"]
                        if fp8:
                            nc.tensor.matmul(
                                ps[:],
                                wt[wname][:, 2 * u : 2 * u + 2, pt * P : (pt + 1) * P],
                                xt[:, 2 * u : 2 * u + 2, x0 : x0 + IB],
                                start=(u == 0), stop=(u == nu - 1),
                                perf_mode=DR,
                            )
                        else:
                            nc.tensor.matmul(
                                ps[:],
                                wt[wname][:, u : u + 1, pt * P : (pt + 1) * P],
                                xt[:, u : u + 1, x0 : x0 + IB],
                                start=(u == 0), stop=(u == nu - 1),
                            )
                        if u == nu - 1:
                            nc.vector.tensor_copy(
                                dst[pt][:, ic * IB : (ic + 1) * IB], ps[:]
                            )
                    yield q

        def gen_proj_v(st):
            fp8 = PROJ_FP8 and st >= LPB
            wt, xt = (w8_sb, xT_sb) if fp8 else (w_sb, xb_sb)
            x0 = st * P  # st < 4 lies inside xb's i<512 window in fp8 mode
            nu = DT // 2 if fp8 else DT
            st8 = {}
            for u in range(nu):
                def q(u=u, st8=st8):
                    if u == 0:
                        st8["ps"] = pp.tile([P, NW], F32, tag="of", name="ps")
                    ps = st8["ps"]
                    if fp8:
                        nc.tensor.matmul(
                            ps[:],
                            xt[:, 2 * u : 2 * u + 2, x0 : x0 + P],
                            wt["v"][:, 2 * u : 2 * u + 2, :],
                            start=(u == 0), stop=(u == nu - 1),
                            perf_mode=DR,
                        )
                    else:
                        nc.tensor.matmul(
                            ps[:],
                            xt[:, u : u + 1, x0 : x0 + P],
                            wt["v"][:, u : u + 1, :],
                            start=(u == 0), stop=(u == nu - 1),
                        )
                    if u == nu - 1:
                        src = ps[:].rearrange("p (h k) -> p h k", k=KS)
                        if st < LPB:
                            nc.vector.tensor_scalar_mul(
                                v0_sb[:, st, :, 0:KS], src, 1.0 / W_SCALE
                            )
                        if PV_FP8:
                            dst = v_sb[:, st // 2, st % 2, :, 0:KS]
                        else:
                            dst = v_sb[:, st, :, 0:KS]
                        nc.vector.tensor_scalar_mul(dst, src, 1.0 / W_SCALE)
                yield q

        bg_queue = []      # ordered quanta
        bg_markers = {}    # name -> index into bg_queue that must be emitted
        bg_pos = [0]

        def pump(n):
            for _ in range(n):
                if bg_pos[0] >= len(bg_queue):
                    return
                bg_queue[bg_pos[0]]()
                bg_pos[0] += 1

        def pump_until(name):
            while bg_pos[0] < bg_markers.get(name, 0):
                pump(1)

        def attention_ib(pr, ib):
            # causal attention + output projection for head pair pr, i-block
            # ib; scores row-packed via tile_position so both heads' K=64
            # matmuls run concurrently on the PE array
            fp8 = PV_FP8 and ib > 0
            nl = (ib + 1) * LPB
            ndiag = ib * LPB  # l-tiles before the diagonal block (off == 0)
            o_ps = [
                po.tile([KSP, IB], F32, tag=f"o{hh}", name=f"o{pr}_{ib}_{hh}")
                for hh in range(2)
            ]
            pe_tiles = {}

            def scores_step(lt):
                off = max(0, (lt - ndiag)) * P
                st_ps = pst.tile([P, 2, IB], F32, tag="st", name="st")
                for hh in range(2):
                    nc.tensor.matmul(
                        st_ps[:, hh, off:IB],
                        kt_sb[pr][hh * KS : (hh + 1) * KS, lt * P : (lt + 1) * P],
                        qt_sb[pr][
                            hh * KS : (hh + 1) * KS,
                            ib * IB + off : (ib + 1) * IB,
                        ],
                        start=True,
                        stop=True,
                        tile_position=(hh * KS, 0),
                    )
                if fp8:
                    if lt % 2 == 0:
                        pe_tiles[lt // 2] = pexp_pool.tile(
                            [P, 2, 2, IB], FP8, tag="pe", name="pe"
                        )
                    pe_t = pe_tiles[lt // 2]
                    pe_sl = pe_t[:, :, lt % 2, :]
                    nc.scalar.activation(
                        pe_sl[:, :, off:IB], st_ps[:, :, off:IB], EXP,
                        scale=EXP_SCALE, bias=bias_sb[:],
                    )
                else:
                    pe_tiles[lt] = pexp_pool.tile(
                        [P, 2, IB], BF16, tag="pe0", name="pe"
                    )
                    pe_sl = pe_tiles[lt][:, :, :]
                    nc.scalar.activation(
                        pe_sl[:, :, off:IB], st_ps[:, :, off:IB], EXP,
                        scale=EXP_SCALE,
                    )
                if lt >= ndiag:  # diagonal 128-block -> triangular mask
                    # on GpSimd (otherwise idle) to keep DVE off the floor
                    for hh in range(2):
                        MASK_ENG.tensor_mul(
                            pe_sl[:, hh, off : off + P],
                            pe_sl[:, hh, off : off + P],
                            mask_sb[:],
                        )

            def pv_step(lt):
                # P@V for l-tile lt (emitted one step late so the exp/mask
                # it depends on is already done when the in-order PE queue
                # reaches it).  Diagonal tiles split into the masked strip
                # and the mask-free tail.
                pump_until(f"v{lt}")  # Tile is program-order: V must be emitted
                off = max(0, (lt - ndiag)) * P
                if fp8:
                    if lt < ndiag:
                        if lt % 2 == 0:
                            return
                        pe_t = pe_tiles[lt // 2]
                        for hh in range(2):  # off-diag pair: DoubleRow K=256
                            nc.tensor.matmul(
                                o_ps[hh][:],
                                v_sb[:, lt // 2, :, 2 * pr + hh, :],
                                pe_t[:, hh, :, :],
                                start=(lt == 1), stop=False,
                                perf_mode=DR,
                            )
                        return
                    pe_t = pe_tiles[lt // 2]
                    for hh in range(2):
                        vsl = v_sb[:, lt // 2, lt % 2, 2 * pr + hh, :]
                        if off + P < IB:
                            nc.tensor.matmul(
                                o_ps[hh][:, off + P : IB],
                                vsl,
                                pe_t[:, hh, lt % 2, off + P : IB],
                                start=(lt == 0), stop=False,
                            )
                        nc.tensor.matmul(
                            o_ps[hh][:, off : off + P],
                            vsl,
                            pe_t[:, hh, lt % 2, off : off + P],
                            start=(lt == 0 and off + P >= IB),
                            stop=(lt == nl - 1),
                        )
                    return
                pe_t = pe_tiles[lt]
                for hh in range(2):
                    vsl = (
                        v0_sb[:, lt, 2 * pr + hh, :]
                        if ib == 0
                        else v_sb[:, lt, 2 * pr + hh, :]
                    )
                    if lt >= ndiag:
                        if off + P < IB:
                            nc.tensor.matmul(
                                o_ps[hh][0 : KS + 1, off + P : IB],
                                vsl,
                                pe_t[:, hh, off + P : IB],
                                start=(lt == 0), stop=False,
                            )
                        nc.tensor.matmul(
                            o_ps[hh][0 : KS + 1, off : off + P],
                            vsl,
                            pe_t[:, hh, off : off + P],
                            start=(lt == 0 and off + P >= IB),
                            stop=(lt == nl - 1),
                        )
                    else:
                        nc.tensor.matmul(
                            o_ps[hh][0 : KS + 1, off:IB],
                            vsl,
                            pe_t[:, hh, off:IB],
                            start=(lt == 0), stop=(lt == nl - 1),
                        )

            def step_slack_ns(step):
                # exp duration of this step minus the PE work emitted in it
                pe = 0.0
                if step < nl:
                    off = max(0, (step - ndiag)) * P
                    exp_ns = 0.833 * 2 * (IB - off) + 260
                    pe += (IB - off) / 2.4  # concurrent score pair
                else:
                    exp_ns = 500.0
                if step >= 2:
                    lt = step - 2
                    off = max(0, (lt - ndiag)) * P
                    if fp8 and lt < ndiag:
                        pe += 2 * IB / 2.4 if lt % 2 == 1 else 0.0
                    else:
                        pe += 2 * (IB - off) / 2.4
                return exp_ns - pe - 150.0

            for step in range(nl + 2):
                if step < nl:
                    scores_step(step)
                # P@V deferred TWO steps: the exp(s)->PV(s)->scores(s+2)->
                # exp(s+2) dependency loop must span >2 exp periods or it,
                # not the scalar engine, sets the attention cadence
                if step >= 2:
                    pv_step(step - 2)
                # fill the remaining exp-period with projection quanta (or
                # dummy fillers) so the PE stays dense and the HAM gate warm.
                # ib=0 blocks get no pumping: the early critical path (proj
                # -> cast -> scores -> exp) must stay free of interlopers,
                # since sem thresholds make later casts wait on any PE work
                # scheduled before them.
                if ib != 0:
                    n = max(0, min(5, int(step_slack_ns(step) / 220)))
                    left = len(bg_queue) - bg_pos[0]
                    pump(n)
                    for _ in range(n - min(n, left)):
                        filler()
                # yield points let the driver interleave this block's first
                # two steps with the previous block's tail, and this block's
                # tail with the next block's head
                if step in (0, 1, nl - 1, nl):
                    yield
            filler()
            for hh in range(2):
                h = 2 * pr + hh
                # bf16 rows for the projection matmul, f32 Z row for
                # exact normalization on the host
                o_bf = osb_pool.tile([KS, IB], BF16, tag="o_bf", name="o_bf")
                nc.vector.tensor_copy(o_bf[:], o_ps[hh][0:KS, :])
                z_sb = osb_pool.tile([1, IB], F32, tag="z_sb", name="z_sb")
                nc.vector.tensor_copy(z_sb[:], o_ps[hh][KS : KS + 1, :])
                nc.sync.dma_start(z[h, ib * IB : (ib + 1) * IB], z_sb[:])
                f_ps = po.tile([KS, IB], F32, tag=f"o{hh}", name="f_ps")
                nc.tensor.matmul(
                    f_ps[:], wkern_sb[:, h, :], o_bf[:], start=True, stop=True
                )
                nc.vector.tensor_copy(
                    outT_sb[:, h, ib * IB : (ib + 1) * IB], f_ps[:]
                )
            nc.sync.dma_start(
                outT[:].rearrange("h k s -> k h s")[
                    :, 2 * pr : 2 * pr + 2, ib * IB : (ib + 1) * IB
                ],
                outT_sb[:, 2 * pr : 2 * pr + 2, ib * IB : (ib + 1) * IB],
            )

        MASK_ENG = nc.gpsimd if MASK_GP else nc.vector

        # Foreground prelude: just enough projection for attn(0,0)'s scores
        # to start, then everything else flows through the background queue,
        # pumped into the attention spine's slack.
        for q in gen_proj_qk(0, 0):
            q()
        for q in gen_proj_v(0):
            q()

        def enqueue(gen, marker=None):
            bg_queue.extend(gen)
            if marker is not None:
                bg_markers[marker] = len(bg_queue)

        bg_markers["v0"] = 0
        enqueue(gen_proj_qk(0, 1), "0,1")
        enqueue(gen_proj_v(1), "v1")
        enqueue(gen_proj_v(2), "v2")
        enqueue(gen_proj_v(3), "v3")
        for st in range(4, 8):
            enqueue(gen_proj_v(st), f"v{st}")
        enqueue(gen_proj_qk(1, 0), "1,0")
        enqueue(gen_proj_qk(1, 1), "1,1")
        enqueue(gen_proj_qk(0, 2))
        for st in range(8, 12):
            enqueue(gen_proj_v(st), f"v{st}")
        bg_markers["0,2"] = len(bg_queue)
        enqueue(gen_proj_qk(1, 2), "1,2")
        enqueue(gen_proj_qk(0, 3))
        for st in range(12, 16):
            enqueue(gen_proj_v(st), f"v{st}")
        bg_markers["0,3"] = len(bg_queue)
        enqueue(gen_proj_qk(1, 3), "1,3")

        # attention spine: alternating pairs, ascending i-blocks — the exp
        # stream runs continuously while the queue drains into its slack.
        # Consecutive blocks overlap: each block's last two pipeline steps
        # interleave with the next block's first two scores steps.
        spine = [(0, 0), (0, 1), (1, 0), (1, 1),
                 (0, 2), (1, 2), (0, 3), (1, 3)]
        prev = None
        for pr, ib in spine:
                pump_until(f"{pr},{ib}")
                g = attention_ib(pr, ib)
                next(g)                      # cur step 0
                if prev is not None:
                    next(prev, None)         # prev tail step nl
                next(g)                      # cur step 1
                if prev is not None:
                    for _ in prev:           # prev last step + outproj
                        pass
                next(g)                      # cur steps 2..nl-1
                prev = g
        if prev is not None:
            for _ in prev:
                pass

    nc.compile()
    return nc


def make_masks():
    # triangular [P, P]: within a diagonal 128-block keep j >= p
    j = np.arange(P)[None, :]
    p = np.arange(P)[:, None]
    return (j >= p).astype(NP_BF16)


def make_in_maps(inputs):
    x = np.asarray(inputs["x"], np.float32)
    Wq = np.asarray(inputs["Wq"], np.float32)
    Wk = np.asarray(inputs["Wk"], np.float32)
    Wv = np.asarray(inputs["Wv"], np.float32)
    kern = np.asarray(inputs["kernel"], np.float32)

    masks = make_masks()
    kern3 = kern.reshape(KS, H, KS)  # [k, h, j]

    def packw(W, hs):
        Wp = W[:, :, hs : hs + NH].transpose(0, 2, 1).reshape(D, NW) * W_SCALE
        # pre-pack to the SBUF [P, DT, NW] layout (row-major flattened)
        return (
            Wp.reshape(DT, P, NW).transpose(1, 0, 2).reshape(P, DT * NW)
            .astype(NP_BF16)
        )

    in_maps = []
    for c in range(NCORES):
        b, hs = c // CORES_PER_B, (c % CORES_PER_B) * NH
        xb = x[b].T  # [D, S]
        in_maps.append(
            {
                "xT0": xb.astype(NP_BF16),
                "xT": np.clip(xb, -240, 240).astype(NP_FP8),
                "wq": packw(Wq, hs),
                "wk": packw(Wk, hs),
                "wv": packw(Wv, hs),
                "wkern": kern3[:, hs : hs + NH, :].reshape(KS, NH * KS)
                .astype(NP_BF16),
                "masks": masks,
            }
        )
    return in_maps


def gather_output(results):
    out = np.zeros((B, S, KS), np.float32)
    for c in range(NCORES):
        b = c // CORES_PER_B
        oT = np.asarray(results[c]["outT"], np.float32)  # [NH, KS, S]
        zz = np.asarray(results[c]["z"], np.float32)     # [NH, S]
        out[b] += (oT / zz[:, None, :]).sum(axis=0).T
    return out


_NC_CACHE = {}


def get_nc():
    if "nc" not in _NC_CACHE:
        _NC_CACHE["nc"] = build_nc()
    return _NC_CACHE["nc"]


def run_hw(inputs, trace=False, **kw):
    from concourse.bass_utils import run_bass_kernel_spmd

    nc = get_nc()
    in_maps = make_in_maps(inputs)
    res = run_bass_kernel_spmd(
        nc, in_maps, list(range(NCORES)), trace=trace, **kw
    )
    return gather_output(res.results), res


def kernel(**inputs) -> np.ndarray:
    out, _ = run_hw(inputs, trace=False)
    return out
